# revision 11
# baseline (speedup 1.0000x reference)
"""Multi-head attention (B=2, S=2048, D=1024, H=16, causal + key/query masks)
on 8 Trainium2 NeuronCores.

Sharding: head-parallel. Core c owns heads {2c, 2c+1}: Wq/Wk/Wv column-sliced,
Wo row-sliced. q/k replicated (each core projects all tokens onto its 2 heads).
Partial outputs (through the Wo row-slice) are summed on the host; bo, the
q_mask row handling and the all-masked-row NaNs are applied on the host.

v2 vs v1: everything bf16 (halves HBM traffic and doubles DVE rates);
k_mask folded into V (V rows zeroed + the ones-column of the AV matmul
carries 0/1 mask) so exp needs no per-k-block bias and runs as one big
ACT instruction; Wo packed as a single K=128 matmul over both heads;
reciprocal via the fast approx DVE op; PSUM evacuation split ACT/DVE.

Device dataflow per core:
  per batch b:
  - QhT/KhT [128feat, 512tok] tiles: projections in transposed layout
    (features on partitions), contraction over D via 8 accumulating bf16
    matmuls; inputs host-pre-arranged so each chunk DMA is one contiguous
    run per partition.
  - Vh [tok, feat(+mask01 col)] via PE transpose of V^T chunks; V rows are
    scaled by k_mask 0/1 during evacuation and the appended mask column
    makes the AV matmul emit (masked) softmax denominators for free.
  - scores^T tiles [128 k-tok, q-range] = KhT^T @ QhT, the 2 heads in
    different PE row-groups (concurrent); exp on ACT with fused 1/8 scale,
    no bias; causal via restricted q-ranges + a 0/1 triangular multiply on
    the diagonal block.
  - P^T tiles feed AV matmuls directly as the moving operand.
  - denominators: fast-approx reciprocal on the psum sums rows, rank-1 PE
    broadcast, fused multiply during pav evacuation; Wo projection as one
    K=128 matmul per 128-row output block; evacuate alternating ACT/DVE,
    DMA out partialT [1024 feat, 4096 tok] bf16 on the second HWDGE ring.
"""
import os
import numpy as np

B, S, D, H = 2, 2048, 1024, 16
NT = B * S            # 4096 tokens
NCORES = 8
HPC = H // NCORES     # heads per core = 2
TCH = 512             # projection token-chunk
QB = 512              # attention q-chunk
NKB = S // 128        # 16 k-blocks per batch
NCH = NT // TCH       # 8 chunks

_CACHE = {}
LAST_EXEC_TIME_NS = None
LAST_RESULTS = None


def _build(causal: bool, reps: int = 1):
    import concourse.bass as bass  # noqa: F401
    from concourse import bacc
    import concourse.tile as tile
    import concourse.mybir as mybir
    from concourse.masks import make_identity

    dt = mybir.dt
    f32, bf16 = dt.float32, dt.bfloat16

    nc = bacc.Bacc("TRN2", target_bir_lowering=False, debug=False,
                   num_devices=NCORES)

    # host pre-arranged: [tch, p, kc*TCH] so each chunk load is one
    # contiguous run per partition
    qT = nc.dram_tensor("qT", [NCH, 128, 8 * TCH], bf16, kind="ExternalInput")
    kT = nc.dram_tensor("kT", [NCH, 128, 8 * TCH], bf16, kind="ExternalInput")
    wqkv = nc.dram_tensor("wqkv", [128, 8 * 384], bf16, kind="ExternalInput")
    bqkv = nc.dram_tensor("bqkv", [128, 3], f32, kind="ExternalInput")
    wo2 = nc.dram_tensor("wo2", [128, D], bf16, kind="ExternalInput")
    km01 = nc.dram_tensor("km01", [128, B * NKB], bf16, kind="ExternalInput")
    sel2d = nc.dram_tensor("sel2d", [2, 128], bf16, kind="ExternalInput")
    partialT = nc.dram_tensor("partialT", [D, NT], bf16, kind="ExternalOutput")

    with tile.TileContext(nc) as tc:
        with tc.tile_pool(name="const", bufs=1) as constp, \
             tc.tile_pool(name="proj", bufs=2 * (S // QB)) as projp, \
             tc.tile_pool(name="qin", bufs=int(os.environ.get("KQIN", "3"))) as qinp, \
             tc.tile_pool(name="vtmp", bufs=2) as vtmpp, \
             tc.tile_pool(name="pt", bufs=int(os.environ.get("KPT", "3"))) as ptp, \
             tc.tile_pool(name="anorm", bufs=3) as anp, \
             tc.tile_pool(name="scale", bufs=2) as scalep, \
             tc.tile_pool(name="outs", bufs=int(os.environ.get("KOUTB", "4"))) as outp, \
             tc.tile_pool(name="psmisc", bufs=2, space="PSUM") as psmisc, \
             tc.tile_pool(name="pss", bufs=2, space="PSUM") as pss, \
             tc.tile_pool(name="psav", bufs=2, space="PSUM") as psav:

            # ---- constants / weights ----
            ident = constp.tile([128, 128], bf16, tag="ident")
            make_identity(nc, ident[:])
            # tri01[kk, qq] = 1.0 if qq >= kk else 0.0 (keep-causal multiplier)
            tri01 = constp.tile([128, 128], bf16, tag="tri01")
            nc.vector.memset(tri01[:], 1.0)
            nc.gpsimd.affine_select(
                out=tri01[:], in_=tri01[:],
                compare_op=mybir.AluOpType.is_ge, fill=0.0, base=0,
                pattern=[[1, 128]], channel_multiplier=-1)
            # sel2[h, p] = 1.0 if p // 64 == h (rank-2 head-broadcast
            # weights, host-provided)
            ones64 = constp.tile([128, 64], bf16, tag="ones64")
            nc.vector.memset(ones64[:], 1.0)

            wqkv_sb = constp.tile([128, 8, 384], bf16, tag="wqkv")
            nc.sync.dma_start(wqkv_sb[:],
                              wqkv.rearrange("p (kc c) -> p kc c", kc=8))
            bqkv_sb = constp.tile([128, 3], f32, tag="bqkv")
            nc.sync.dma_start(bqkv_sb[:], bqkv[:, :])
            wo2_sb = constp.tile([128, D], bf16, tag="wo2")
            nc.sync.dma_start(wo2_sb[:], wo2[:, :])
            km01_sb = constp.tile([128, B * NKB], bf16, tag="km01")
            nc.sync.dma_start(km01_sb[:], km01[:, :])
            # fp32 copy: tensor_scalar scalar operands must be fp32
            km01f = constp.tile([128, B * NKB], f32, tag="km01f")
            nc.vector.tensor_copy(km01f[:], km01_sb[:])

            NG = S // QB          # 512-token groups per batch

            for _rep in range(reps):
              tiles = {}   # b -> (QhTg, KhTg, Vhg)

              def alloc_group_tiles(b):
                  QhTg, KhTg, Vhg = [], [], []
                  for g in range(NG):
                      QhT = projp.tile([128, QB], bf16, tag="QhT",
                                       name=f"QhT_{b}_{g}")
                      KhT = projp.tile([128, QB], bf16, tag="KhT",
                                       name=f"KhT_{b}_{g}")
                      # Vh: per 128-token block kk (0..3) and head h: [128,65]
                      # (64 feats + mask01 col) at free offset (kk*2+h) * 65
                      Vh = projp.tile([128, (QB // 128) * 2 * 65], bf16,
                                      tag="Vh", name=f"Vh_{b}_{g}")
                      # mask01 columns for this group's 4 k-blocks, both heads
                      kmg = km01_sb[:, b * NKB + g * 4:b * NKB + g * 4 + 4] \
                          .rearrange("p (k o) -> p k o", o=1)
                      vh4 = Vh[:].rearrange("p (kk h f) -> p kk h f",
                                            kk=QB // 128, h=2)
                      for h in range(2):
                          nc.vector.tensor_copy(vh4[:, :, h, 64:65], kmg)
                      QhTg.append(QhT)
                      KhTg.append(KhT)
                      Vhg.append(Vh)
                  return QhTg, KhTg, Vhg

              def proj_group(b, g):
                  QhT, KhT, Vh = (t[g] for t in tiles[b])
                  tg = b * (S // TCH) + g
                  qin = qinp.tile([128, 8, TCH], bf16, tag="qin")
                  nc.sync.dma_start(
                      qin[:], qT[tg].rearrange("p (kc t) -> p kc t", kc=8))
                  kin = qinp.tile([128, 8, TCH], bf16, tag="kin")
                  nc.sync.dma_start(
                      kin[:], kT[tg].rearrange("p (kc t) -> p kc t", kc=8))

                  for j, (src_, dstT) in enumerate(((qin, QhT), (kin, KhT))):
                      ps = psmisc.tile([128, TCH], f32, tag="ps")
                      for kc in range(8):
                          nc.tensor.matmul(
                              ps[:], wqkv_sb[:, kc, j * 128:(j + 1) * 128],
                              src_[:, kc, :],
                              start=(kc == 0), stop=(kc == 7))
                      nc.vector.tensor_scalar_add(dstT[:], ps[:],
                                                  bqkv_sb[:, j:j + 1])
                  # V projection (from k), then transpose to [tok, feat]
                  ps = psmisc.tile([128, TCH], f32, tag="ps")
                  for kc in range(8):
                      nc.tensor.matmul(ps[:], wqkv_sb[:, kc, 256:384],
                                       kin[:, kc, :],
                                       start=(kc == 0), stop=(kc == 7))
                  vtmp = vtmpp.tile([128, TCH], bf16, tag="vtmp")
                  nc.vector.tensor_scalar_add(vtmp[:], ps[:], bqkv_sb[:, 2:3])
                  for kk in range(TCH // 128):
                      pvT = psmisc.tile([128, 128], bf16, tag="ps")
                      nc.tensor.transpose(
                          pvT[:], vtmp[:, kk * 128:(kk + 1) * 128], ident[:])
                      # evacuate with fused k_mask 0/1 row scaling
                      dst = Vh[:, kk * 130:(kk + 1) * 130] \
                          .rearrange("p (h f) -> p h f", h=2)[:, :, 0:64]
                      nc.vector.tensor_scalar_mul(
                          dst, pvT[:].rearrange("p (h f) -> p h f", h=2),
                          km01f[:, b * NKB + g * 4 + kk:
                                b * NKB + g * 4 + kk + 1])

              def norm_and_wo(b, qc, pav):
                  # 1/rowsum (both heads) -> rank-1 PE broadcast -> fused
                  # normalize during pav evacuation -> Wo in one K=128 matmul
                  rec2 = scalep.tile([128, QB], f32, tag="rec2",
                                     name=f"rec2_{b}_{qc}")
                  rec2b = scalep.tile([128, QB], bf16, tag="rec2b",
                                      name=f"rec2b_{b}_{qc}")
                  psb = psmisc.tile([128, QB], f32, tag="ps",
                                    name=f"psb_{b}_{qc}")
                  for h in range(HPC):
                      if os.environ.get("KREC", "exact") == "approx":
                          nc.vector.reciprocal_approx_fast(
                              out=rec2[64 * h:64 * h + 1],
                              in_=pav[h][64:65, :])
                      else:
                          nc.vector.reciprocal(rec2[64 * h:64 * h + 1],
                                               pav[h][64:65, :])
                      nc.vector.tensor_copy(rec2b[64 * h:64 * h + 1],
                                            rec2[64 * h:64 * h + 1])
                      nc.tensor.matmul(psb[64 * h:64 * (h + 1)],
                                       ones64[64 * h:64 * h + 1, :],
                                       rec2b[64 * h:64 * h + 1],
                                       start=True, stop=True)
                  psbs = scalep.tile([128, QB], f32, tag="psbs",
                                     name=f"psbs_{b}_{qc}")
                  nc.vector.tensor_copy(psbs[:], psb[:])
                  an = anp.tile([128, QB], bf16, tag="anorm",
                                name=f"anorm_{b}_{qc}")
                  for h in range(HPC):
                      nc.vector.tensor_mul(an[h * 64:(h + 1) * 64, :],
                                           pav[h][0:64, :],
                                           psbs[h * 64:(h + 1) * 64, :])
                  for fb in range(D // 128):
                      po = psmisc.tile([128, QB], f32, tag="ps")
                      nc.tensor.matmul(
                          po[:], wo2_sb[:, fb * 128:(fb + 1) * 128], an[:],
                          start=True, stop=True)
                      osb = outp.tile([128, QB], bf16, tag="outs")
                      if fb % 2 == 0:
                          nc.scalar.copy(osb[:], po[:])
                      else:
                          nc.vector.tensor_copy(osb[:], po[:])
                      nc.scalar.dma_start(
                          partialT[fb * 128:(fb + 1) * 128,
                                   b * S + qc * QB:b * S + (qc + 1) * QB],
                          osb[:])

              def attn_chunk(b, qc):
                  QhTg, KhTg, Vhg = tiles[b]
                  kb_max = (qc * (QB // 128) + (QB // 128) - 1) if causal \
                      else NKB - 1
                  pav = [psav.tile([65, QB], f32, tag="psav",
                                   name=f"pav_{b}_{qc}_{hh}")
                         for hh in range(HPC)]
                  for kb in range(kb_max + 1):
                      qls = max(0, kb * 128 - qc * QB) if causal else 0
                      kg, kk = kb // (QB // 128), kb % (QB // 128)
                      ps = pss.tile([128, 2 * QB], f32, tag="pss")
                      for h in range(HPC):
                          nc.tensor.matmul(
                              ps[:, h * QB + qls:(h + 1) * QB],
                              KhTg[kg][h * 64:(h + 1) * 64,
                                       kk * 128:(kk + 1) * 128],
                              QhTg[qc][h * 64:(h + 1) * 64, qls:QB],
                              start=True, stop=True)
                      pt = ptp.tile([128, 2 * QB], bf16, tag="pt")
                      if qls == 0:
                          nc.scalar.activation(
                              pt[:, 0:2 * QB], ps[:, 0:2 * QB],
                              mybir.ActivationFunctionType.Exp, scale=0.125)
                      else:
                          for h in range(HPC):
                              rg = slice(h * QB + qls, (h + 1) * QB)
                              nc.scalar.activation(
                                  pt[:, rg], ps[:, rg],
                                  mybir.ActivationFunctionType.Exp,
                                  scale=0.125)
                      if causal and kb >= qc * (QB // 128):
                          for h in range(HPC):
                              dg = slice(h * QB + qls, h * QB + qls + 128)
                              nc.vector.tensor_mul(pt[:, dg], pt[:, dg],
                                                   tri01[:])
                      off = kk * 130
                      for h in range(HPC):
                          nc.tensor.matmul(
                              pav[h][:, qls:QB],
                              Vhg[kg][:, off + h * 65:off + (h + 1) * 65],
                              pt[:, h * QB + qls:(h + 1) * QB],
                              start=(kb == 0), stop=(kb == kb_max))
                  norm_and_wo(b, qc, pav)

              # projections for BOTH batches first (input DMA streams without
              # ever waiting on attention), then attention for both batches
              if os.environ.get("KSPLIT", "1") == "1":
                  for b in range(B):
                      tiles[b] = alloc_group_tiles(b)
                      for g in range(NG):
                          proj_group(b, g)
                  for b in range(B):
                      for qc in range(S // QB):
                          attn_chunk(b, qc)
              else:
                  for b in range(B):
                      tiles[b] = alloc_group_tiles(b)
                      for g in range(NG):
                          proj_group(b, g)
                      for qc in range(S // QB):
                          attn_chunk(b, qc)

    nc.compile()
    return nc


def _prep(q, k, k_mask, Wq, bq, Wk, bk, Wv, bv, Wo):
    import ml_dtypes
    bf = ml_dtypes.bfloat16
    f = np.float32
    q2 = np.asarray(q, dtype=f).reshape(NT, D)
    k2 = np.asarray(k, dtype=f).reshape(NT, D)
    # [tch, p, kc*TCH]: chunk tch, partition p reads one contiguous run
    qTn = np.ascontiguousarray(
        q2.T.reshape(8, 128, NCH, TCH).transpose(2, 1, 0, 3)
        .reshape(NCH, 128, 8 * TCH).astype(bf))
    kTn = np.ascontiguousarray(
        k2.T.reshape(8, 128, NCH, TCH).transpose(2, 1, 0, 3)
        .reshape(NCH, 128, 8 * TCH).astype(bf))
    Wq, Wk, Wv, Wo = (np.asarray(x, dtype=f) for x in (Wq, Wk, Wv, Wo))
    bq, bk, bv = (np.asarray(x, dtype=f) for x in (bq, bk, bv))
    km = np.asarray(k_mask)
    km01n = np.ascontiguousarray(
        np.where(km.reshape(B * NKB, 128) == 0, 0.0, 1.0).T.astype(bf))
    sel2n = np.zeros((2, 128), dtype=bf)
    sel2n[0, 0:64] = 1
    sel2n[1, 64:128] = 1

    in_maps = []
    for c in range(NCORES):
        hc = slice(128 * c, 128 * (c + 1))
        wqkv_c = np.concatenate([Wq[hc].T, Wk[hc].T, Wv[hc].T], axis=1)
        wqkv_c = np.ascontiguousarray(
            wqkv_c.reshape(8, 128, 384).transpose(1, 0, 2)
            .reshape(128, -1).astype(bf))
        bqkv_c = np.ascontiguousarray(
            np.stack([bq[hc], bk[hc], bv[hc]], axis=1))
        wo2_c = np.ascontiguousarray(Wo[:, hc].T.astype(bf))
        in_maps.append({
            "qT": qTn, "kT": kTn, "wqkv": wqkv_c, "bqkv": bqkv_c,
            "wo2": wo2_c, "km01": km01n, "sel2d": sel2n,
        })
    return in_maps


def kernel(q, k, q_mask, k_mask, Wq, bq, Wk, bk, Wv, bv, Wo, bo,
           causal_attention):
    global LAST_EXEC_TIME_NS, LAST_RESULTS
    from concourse.bass_utils import run_bass_kernel_spmd

    causal = bool(int(np.asarray(causal_attention)))
    if causal not in _CACHE:
        _CACHE[causal] = _build(causal)
    nc = _CACHE[causal]

    in_maps = _prep(q, k, k_mask, Wq, bq, Wk, bk, Wv, bv, Wo)
    trace = os.environ.get("KERNEL_TRACE", "0") == "1"
    try:
        res = run_bass_kernel_spmd(nc, in_maps, list(range(NCORES)),
                                   trace=trace)
    except ModuleNotFoundError:
        # NTFF profiling hook unavailable in this container build
        res = run_bass_kernel_spmd(nc, in_maps, list(range(NCORES)),
                                   trace=False)
    LAST_EXEC_TIME_NS = res.exec_time_ns
    LAST_RESULTS = res

    acc = res.results[0]["partialT"].astype(np.float32)
    for c in range(1, NCORES):
        acc += res.results[c]["partialT"].astype(np.float32)
    bo32 = np.asarray(bo, dtype=np.float32)
    out = (acc.T + bo32[None, :]).reshape(B, S, D)
    km = np.asarray(k_mask)
    if causal:
        visible = np.cumsum(km, axis=1)          # of unmasked k with k <= q
    else:
        visible = np.broadcast_to(km.sum(axis=1)[:, None], (B, S))
    out[visible == 0] = np.nan  # all-visible-k-masked rows: softmax is NaN
    qm0 = np.asarray(q_mask) == 0
    out[qm0] = bo32  # reference: attn rows with q_mask==0 -> out = bo exactly
    return np.ascontiguousarray(out)
